# revision 18
# baseline (speedup 1.0000x reference)
"""CompressAttn Trainium2 Bass kernel (v2).

Problem: compressed-block attention.
  B=2, N=4096, QH=32, KH=2, D=VD=128, KSZ=32, STRIDE=16, M=255 blocks.
  kc[b,m,h,:] = sum_i w_k[i] * (k[b,16m+i,h,:] + pe_k[i,:])   (same for v)
  out = softmax(q @ kc^T * D^-0.5, causal-banded mask) @ vc, zero for n < 31.

Sharding: 8 cores = (batch b in {0,1}) x (query-head quarter hq in {0..3}).
Each core handles 8 query heads that share a single KV head (g = hq//2), so
K/V compression is done once per core.  No collectives needed; host gathers.

v2 structure (vs v1): the staircase mask is ADDED into the QK psum by the
tensor engine (selection-identity stationary x NEG-mask moving), so the
Scalar engine runs exactly one exp per (head, 512-query block) and the
Vector engine runs exactly one reciprocal + one broadcast normalization
multiply per block.  Outputs are written bf16, one contiguous DMA per head
([128 part, 32 tile, 128 vd] layout), and the host untangles the tiling.

Per-core device pipeline:
  1. Compression via banded matmul (bf16): psum P_a[t] partial sums ->
     kcT[d,m] (+bias) bf16, vcT -> PE-transpose -> vca=[vc|1|0] bf16.
  2. Per (head, block b): QK psum tile [128, 1024] = two 512-col halves
     (m-chunk 0 / m-chunk 1); per chunk a matmul kcT_chunk^T @ qT then an
     accumulated mask matmul (start=False) adding -16384 above the causal
     staircase.  One exp (scale=D^-0.5) over the whole tile -> eT bf16.
  3. PV psum tile [128, 1024]: per 128-query subtile tt: eT_tile^T @ [vc|1|0]
     at col 512*pr + 130*j (accumulating both m-chunks); ones column gives
     the softmax denominator at cols {128, 258, 640, 770}.
  4. One reciprocal [128,4] + one tensor_tensor mul (rc broadcast along vd
     via stride-0 AP) -> per-head bf16 output tile; one DMA per head.
"""

import ml_dtypes
import numpy as np

import concourse.bacc as bacc
import concourse.mybir as mybir
import concourse.tile as tile
from concourse.bass_utils import run_bass_kernel_spmd

# Problem geometry (hardcoded per contest rules).
B, N, QH, KH, D, VD = 2, 4096, 32, 2, 128, 128
KSZ, STRIDE = 32, 16
M = (N - KSZ) // STRIDE + 1          # 255 compressed blocks (m = 0..254)
HPC = QH // 4                         # 8 query heads per core
NBLK = N // 512                       # 8 query blocks of 512
SM = float(D) ** -0.5
NEGM = -16384.0                       # mask add; exp(SM*(-16384+s)) == 0

F32 = mybir.dt.float32
BF16 = mybir.dt.bfloat16


def build_program():
    nc = bacc.Bacc("TRN2", target_bir_lowering=False, debug=False)

    qT_d = nc.dram_tensor("qT", [HPC, D, N], BF16, kind="ExternalInput")
    k_d = nc.dram_tensor("kk", [N, D], BF16, kind="ExternalInput")
    v_d = nc.dram_tensor("vv", [N, D], BF16, kind="ExternalInput")
    w01k_d = nc.dram_tensor("w01k", [128, 16], BF16, kind="ExternalInput")
    w01v_d = nc.dram_tensor("w01v", [128, 16], BF16, kind="ExternalInput")
    bk_d = nc.dram_tensor("biask", [128, 1], F32, kind="ExternalInput")
    bv_d = nc.dram_tensor("biasv", [128, 1], F32, kind="ExternalInput")
    mv_d = nc.dram_tensor("maskv", [8, 128, 512], BF16, kind="ExternalInput")
    id_d = nc.dram_tensor("ident", [128, 128], F32, kind="ExternalInput")
    idb_d = nc.dram_tensor("identb", [128, 128], BF16, kind="ExternalInput")
    ones_d = nc.dram_tensor("ones1", [128, 2], BF16, kind="ExternalInput")
    o_d = nc.dram_tensor("o", [HPC, 128, N // 128, VD], BF16,
                         kind="ExternalOutput")

    with tile.TileContext(nc) as tc:
        with tc.tile_pool(name="consts", bufs=1) as cp:
            w01k = cp.tile([128, 16], BF16)
            w01v = cp.tile([128, 16], BF16)
            biask = cp.tile([128, 1], F32)
            biasv = cp.tile([128, 1], F32)
            maskv = cp.tile([128, 8 * 512], BF16)
            ident = cp.tile([128, 128], F32)
            identb = cp.tile([128, 128], BF16)
            ktile = cp.tile([128, 32 * 128], BF16)
            vtile = cp.tile([128, 32 * 128], BF16)
            kcT = cp.tile([128, 256], BF16)       # [d, m] (col 255 zero pad)
            vcT = cp.tile([128, 256], F32)        # [d, t] staging
            vca0 = cp.tile([128, 130], BF16)      # [m 0:128,   vc|1|0]
            vca1 = cp.tile([128, 130], BF16)      # [m 128:255, vc|1|0]
            q0 = cp.tile([128, N], BF16)          # head 0 qT (early load)

            # compression-critical loads on the SP queue; first q head and
            # mask constants on the Activation HWDGE queue (idle at start)
            nc.sync.dma_start(w01k[:, :], w01k_d.ap())
            nc.sync.dma_start(w01v[:, :], w01v_d.ap())
            nc.sync.dma_start(
                ktile[:, :].rearrange("p (c d) -> p c d", c=32),
                k_d.ap().rearrange("(c r) d -> r c d", r=128),
            )
            nc.sync.dma_start(
                vtile[:, :].rearrange("p (c d) -> p c d", c=32),
                v_d.ap().rearrange("(c r) d -> r c d", r=128),
            )
            nc.scalar.dma_start(biask[:, :], bk_d.ap())
            nc.scalar.dma_start(biasv[:, :], bv_d.ap())
            nc.scalar.dma_start(ident[:, :], id_d.ap())
            nc.scalar.dma_start(identb[:, :], idb_d.ap())
            mv3 = maskv[:, :].rearrange("p (j n) -> p j n", j=8)
            nc.scalar.dma_start(mv3[:, 0:2], mv_d.ap().rearrange(
                "j p n -> p j n")[:, 0:2])
            nc.scalar.dma_start(q0[:, :], qT_d.ap()[0])
            nc.scalar.dma_start(mv3[:, 2:8], mv_d.ap().rearrange(
                "j p n -> p j n")[:, 2:8])

            # ---- compression ----
            with tc.tile_pool(name="ppsum", bufs=1, space="PSUM") as pp:
                # free layout (t, a): pkT[d, 2t+a] = P_a[t]
                pkT = pp.tile([128, 512], F32)
                pvT = pp.tile([128, 512], F32)
                tpA = pp.tile([128, 128], F32)
                tpB = pp.tile([128, 128], F32)
                # k first (so kcT is ready before v compression finishes)
                for c in range(32):
                    nc.tensor.matmul(
                        pkT[:, 16 * c : 16 * c + 16],
                        ktile[:, 128 * c : 128 * (c + 1)],
                        w01k[:, :],
                        start=True, stop=True,
                    )
                # kcT[d,m] = P0[m] + P1[m+1] + bias_k[d]
                pk3 = pkT[:, :].rearrange("p (t a) -> p t a", a=2)
                # (walrus: only one PSUM input per DVE op -> two steps)
                nc.vector.tensor_scalar_add(kcT[:, 0:M], pk3[:, 0:M, 0], biask[:, 0:1])
                nc.vector.tensor_add(kcT[:, 0:M], kcT[:, 0:M], pk3[:, 1 : M + 1, 1])
                nc.vector.memset(kcT[:, M:256], 0.0)
                for c in range(32):
                    nc.tensor.matmul(
                        pvT[:, 16 * c : 16 * c + 16],
                        vtile[:, 128 * c : 128 * (c + 1)],
                        w01v[:, :],
                        start=True, stop=True,
                    )
                pv3 = pvT[:, :].rearrange("p (t a) -> p t a", a=2)
                nc.vector.tensor_scalar_add(vcT[:, 0:M], pv3[:, 0:M, 0], biasv[:, 0:1])
                nc.vector.tensor_add(vcT[:, 0:M], vcT[:, 0:M], pv3[:, 1 : M + 1, 1])
                nc.vector.memset(vcT[:, M : M + 1], 0.0)
                # transpose vcT -> natural vc, build [vc|1|0]
                nc.tensor.transpose(tpA[:, :], vcT[:, 0:128], ident[:, :])
                nc.tensor.transpose(tpB[:, :], vcT[:, 128:256], ident[:, :])
                nc.vector.tensor_copy(vca0[:, 0:128], tpA[:, :])
                nc.vector.tensor_copy(vca1[:, 0:128], tpB[:, :])
                nc.scalar.dma_start(vca0[:, 128:130], ones_d.ap())
                nc.scalar.dma_start(vca1[:, 128:130], ones_d.ap())

            # ---- attention ----
            with (
                tc.tile_pool(name="qp", bufs=3) as qp,
                tc.tile_pool(name="ep", bufs=4) as ep,
                tc.tile_pool(name="op", bufs=2) as op,
                tc.tile_pool(name="rp", bufs=8) as rp,
                tc.tile_pool(name="ps", bufs=4, space="PSUM") as ps,
            ):
                for h in range(HPC):
                    if h == 0:
                        qTh = q0
                    else:
                        qTh = qp.tile([128, N], BF16, tag="qTh")
                        nc.sync.dma_start(qTh[:, :], qT_d.ap()[h])
                    o_head = op.tile([128, N], BF16, tag="o")
                    for b in range(NBLK):
                        mr = min(32 * b + 31, M)      # visible m count
                        c1r = mr - 128
                        nchunk = 1 if c1r <= 0 else 2
                        qs = qTh[:, 512 * b : 512 * (b + 1)]

                        # QK psum: [128, 1024] = [chunk0 | chunk1]; both
                        # chunks always compute all 128 rows (junk rows past
                        # the visible count are never read by PV).  The SAME
                        # tile is reused as the PV accumulator below (PV
                        # depends on exp, exp is the last score reader, so
                        # the reuse costs nothing and doubles ring depth).
                        sT = ps.tile([128, 1024], F32, tag="sT")
                        for c in range(nchunk):
                            v = b - 4 * c           # mask variant
                            has_mask = 32 * v - 1 < 128
                            nc.tensor.matmul(
                                sT[:, 512 * c : 512 * c + 512],
                                kcT[:, 128 * c : 128 * c + 128],
                                qs,
                                start=True, stop=not has_mask,
                            )
                            if has_mask:
                                # rows below the band add 0 (free: matmul
                                # cost is column-count only); base must be
                                # 32-aligned so start at 0
                                be = min(128, 32 * v + 32)
                                nc.tensor.matmul(
                                    sT[0:be, 512 * c : 512 * c + 512],
                                    identb[:, 0:be],
                                    maskv[:, 512 * v : 512 * v + 512],
                                    start=False, stop=True,
                                )
                        # one exp per block
                        ecols = 512 * nchunk
                        eT = ep.tile([128, 1024], BF16, tag="eT")
                        nc.scalar.activation(
                            eT[:, 0:ecols], sT[:, 0:ecols],
                            mybir.ActivationFunctionType.Exp, scale=SM,
                        )
                        # PV accumulates into the score tile (see above):
                        # col 512*pr + 130*j, tt = 2*pr + j
                        pvt = sT
                        for pr in range(2):
                            for j in range(2):
                                tt = 2 * pr + j
                                t = 4 * b + tt
                                K = 8 * t + 7
                                c0k = min(K, 128)
                                c1k = K - 128
                                out_ap = pvt[:, 512 * pr + 130 * j :
                                             512 * pr + 130 * j + 130]
                                nc.tensor.matmul(
                                    out_ap,
                                    eT[0:c0k, 128 * tt : 128 * (tt + 1)],
                                    vca0[0:c0k, :],
                                    start=True, stop=(c1k <= 0),
                                )
                                if c1k > 0:
                                    nc.tensor.matmul(
                                        out_ap,
                                        eT[0:c1k, 512 + 128 * tt : 512 + 128 * (tt + 1)],
                                        vca1[0:c1k, :],
                                        start=False, stop=True,
                                    )
                        # denominators at cols 128 + 512*pr + 130*j
                        pvt3 = pvt[:, :].rearrange("p (pr x) -> p pr x", pr=2)
                        den = pvt3[:, :, 128:259:130]  # [128, 2, 2]
                        rc = rp.tile([128, 4], F32, tag="rc")
                        r4 = rc[:, :].rearrange("p (a b) -> p a b", a=2)
                        if b == 0:
                            rtmp = rp.tile([128, 4], F32, tag="rtmp")
                            t4 = rtmp[:, :].rearrange("p (a b) -> p a b", a=2)
                            nc.vector.tensor_scalar_add(t4, den, 1e-30)
                            nc.vector.reciprocal(r4, t4)
                        else:
                            nc.vector.reciprocal(r4, den)
                        # one normalization mul: [128, (pr, j, vd)] * rc bcast
                        pv4 = pvt3[:, :, 0:260].rearrange(
                            "p pr (j x) -> p pr j x", j=2
                        )[:, :, :, 0:128]             # [128, 2, 2, 128]
                        rcb = rc[:, :].rearrange(
                            "p (a b) -> p a b", a=2
                        ).unsqueeze(3).broadcast_to([128, 2, 2, 128])
                        dst = o_head[:, 512 * b : 512 * (b + 1)].rearrange(
                            "p (pr j x) -> p pr j x", pr=2, j=2
                        )
                        nc.vector.tensor_mul(dst, pv4, rcb)
                        if b in (3, 7):
                            half = (b - 3) // 4
                            nc.scalar.dma_start(
                                o_d.ap()[h].rearrange("p t v -> p (t v)")[
                                    :, 2048 * half : 2048 * half + 2048],
                                o_head[:, 2048 * half : 2048 * half + 2048],
                            )
    nc.compile()
    return nc


def make_consts(w_k, pe_k, w_v, pe_v):
    """Host-side constant tensors fed to every core."""
    f = np.float32
    w01k = np.zeros((128, 16), f)
    w01v = np.zeros((128, 16), f)
    for r in range(128):
        j = r // 16
        s = r % 16
        for a in range(2):
            # column layout (j, a): col = 2*j + a, matching psum (t, a)
            w01k[r, 2 * j + a] = w_k[16 * a + s]
            w01v[r, 2 * j + a] = w_v[16 * a + s]
    biask = (w_k[:, None] * pe_k).sum(0).astype(f)[:, None]  # [128,1]
    biasv = (w_v[:, None] * pe_v).sum(0).astype(f)[:, None]
    # mask variant v (= b - 4*chunk), row p (chunk-local m), col n' (block-
    # local query): masked iff n' < 16*p + 31 - 512*v.
    maskv = np.zeros((8, 128, 512), f)
    for v in range(8):
        for p in range(128):
            lo = 16 * p + 31 - 512 * v
            if lo > 0:
                maskv[v, p, : min(lo, 512)] = NEGM
    ident = np.eye(128, dtype=f)
    return {
        "w01k": np.ascontiguousarray(w01k).astype(ml_dtypes.bfloat16),
        "w01v": np.ascontiguousarray(w01v).astype(ml_dtypes.bfloat16),
        "biask": np.ascontiguousarray(biask),
        "biasv": np.ascontiguousarray(biasv),
        "maskv": maskv.astype(ml_dtypes.bfloat16),
        "ident": ident,
        "identb": ident.astype(ml_dtypes.bfloat16),
        "ones1": np.hstack([np.ones((128, 1)), np.zeros((128, 1))]).astype(
            ml_dtypes.bfloat16),
    }


def make_in_map(q, k, v, consts, core):
    b, hq = core // 4, core % 4
    g = hq // 2
    qT = np.ascontiguousarray(
        q[b, :, 8 * hq : 8 * (hq + 1), :].transpose(1, 2, 0)
    ).astype(ml_dtypes.bfloat16)  # [8, D, N]
    return {
        "qT": qT,
        "kk": np.ascontiguousarray(k[b, :, g, :]).astype(ml_dtypes.bfloat16),
        "vv": np.ascontiguousarray(v[b, :, g, :]).astype(ml_dtypes.bfloat16),
        **consts,
    }


_CACHE = {}


def _compiled():
    if "nc" not in _CACHE:
        _CACHE["nc"] = build_program()
    return _CACHE["nc"]


def kernel(q, k, v, w_k, pe_k, w_v, pe_v, _trace=False, _trace_kwargs=None):
    q = np.asarray(q, np.float32)
    k = np.asarray(k, np.float32)
    v = np.asarray(v, np.float32)
    consts = make_consts(
        np.asarray(w_k, np.float32), np.asarray(pe_k, np.float32),
        np.asarray(w_v, np.float32), np.asarray(pe_v, np.float32),
    )
    nc = _compiled()
    in_maps = [make_in_map(q, k, v, consts, c) for c in range(8)]
    kw = {}
    if _trace:
        kw = {"trace": True, **(_trace_kwargs or {})}
    res = run_bass_kernel_spmd(nc, in_maps, core_ids=list(range(8)), **kw)
    out = np.empty((B, N, QH, VD), np.float32)
    for c in range(8):
        b, hq = c // 4, c % 4
        # o: [HPC, 128 p, 32 t, VD]; query n = 128*t + p
        oc = np.asarray(res.results[c]["o"], dtype=np.float32)
        out[b, :, 8 * hq : 8 * (hq + 1), :] = (
            oc.transpose(2, 1, 0, 3).reshape(N, HPC, VD)
        )
    _CACHE["last_result"] = res
    return out
